# revision 9
# baseline (speedup 1.0000x reference)
"""Trainium2 Bass kernel for nn_CrossAttention_51539607552970.

Sharding: 8 cores = 2 (batch) x 4 (GQA kv-head groups). Each core computes
4 query heads + its single kv head for one batch element, producing a
partial output (its head-group's contribution through wo); the host sums
the 4 partials per batch element (tensor-parallel unshard).

v4:
- Consolidated, ordered DMAs (3-D DRAM tensors); input DMAs issued before
  const setup so the wire starts at t~1us.
- fp16 activations (kT / v / xq / es).
- Softmax denominator accumulated across key-chunks on the DVE (fp16 2x)
  + ONE ones-matmul per (head, q-block) instead of 16.
- Paired [128,1024] 2-bank PSUM tiles halve Scalar exp count.
- PSUM: 4-slot [128,512] "att" ring + 2-slot [128,1024] "big" ring, so the
  next block's P@V accumulators never wait on the previous block's
  normalize chain; score matmuls for the next q-block are pre-emitted
  before wo (software pipelining across block boundaries).
- Output DMAs on the GpSimd software-DGE queue so their semaphore waits
  don't head-of-line-block the input stream on the Sync queue.
"""

import sys

sys.path.insert(0, "/opt/trn_rl_repo")

import numpy as np

import concourse.bass as bass
import concourse.mybir as mybir
import concourse.tile as tile
from concourse import bacc
from concourse.bass_utils import run_bass_kernel_spmd
from concourse.masks import make_identity

F32 = mybir.dt.float32
F32R = mybir.dt.float32r
F16 = mybir.dt.float16
AF = mybir.ActivationFunctionType
OP = mybir.AluOpType

# Problem constants (hardcoded per contract).
B, S, L = 2, 2048, 2048
H, KVH, D = 16, 4, 128
HID = H * D
EPS = 1e-6
SCALE = 1.0 / np.sqrt(D)

NH = 4           # query heads per core
P = 128          # partitions
HC = HID // P    # 16 hid chunks
KC = L // P      # 16 key chunks
NB = 4           # 512-wide blocks per 2048 (both keys and queries)

_compiled = None


def _build():
    nc = bacc.Bacc("TRN2", num_devices=8)

    xT = nc.dram_tensor("xT", [P, HC, S], F32R, kind="ExternalInput")
    cT = nc.dram_tensor("cT", [P, HC, L], F32R, kind="ExternalInput")
    wq = nc.dram_tensor("wq", [P, HC, NH * D], F32R, kind="ExternalInput")
    wkv = nc.dram_tensor("wkv", [P, HC, 2 * D], F32R, kind="ExternalInput")
    wo = nc.dram_tensor("wo", [P, NH, HID], F32R, kind="ExternalInput")
    nqw = nc.dram_tensor("nqw", [P, 1], F32, kind="ExternalInput")
    nkw = nc.dram_tensor("nkw", [P, 1], F32, kind="ExternalInput")
    out = nc.dram_tensor("out", [S, HID], F32, kind="ExternalOutput")

    with nc.allow_low_precision(reason="f32r/f16 matmul input rounding"), \
         tile.TileContext(nc) as tc:
        with tc.tile_pool(name="consts", bufs=1) as consts, \
             tc.tile_pool(name="weights", bufs=1) as weights, \
             tc.tile_pool(name="stream", bufs=12) as stream, \
             tc.tile_pool(name="kv", bufs=1) as kvpool, \
             tc.tile_pool(name="xqt", bufs=1) as xqtpool, \
             tc.tile_pool(name="small", bufs=1) as small, \
             tc.tile_pool(name="esbp", bufs=3) as esbp, \
             tc.tile_pool(name="accp", bufs=2) as accp, \
             tc.tile_pool(name="attnp", bufs=1) as attnp, \
             tc.tile_pool(name="outp", bufs=2) as outp, \
             tc.tile_pool(name="psum", bufs=1, space="PSUM") as psum:

            # ---- resident weights + persistent activations ----
            wq_sb = weights.tile([P, HC * NH * D], F32R)
            wkv_sb = weights.tile([P, HC * 2 * D], F32R)
            wo_sb = weights.tile([P, NH * HID], F32R)
            kT_sb = kvpool.tile([P, L], F16)              # [D, keys]
            v_sb = kvpool.tile([P, KC * D], F16)          # kt-th block = [keys, D]
            xqT_list = [xqtpool.tile([P, S], F16, name=f"xqT{h}")
                        for h in range(NH)]
            nqw_sb = consts.tile([P, 1], F32)
            nkw_sb = consts.tile([P, 1], F32)

            def dma_stream(src, blk):
                tiles = []
                for hp in range(HC // 2):
                    t = stream.tile([P, 2 * 512], F32R, name="st", tag="stream")
                    nc.sync.dma_start(
                        t[:], src[:, 2 * hp:2 * hp + 2,
                                  blk * 512:(blk + 1) * 512])
                    tiles.append(t)
                return tiles

            # ---- first input DMAs before anything else ----
            nc.sync.dma_start(nqw_sb[:], nqw[:])
            nc.sync.dma_start(nkw_sb[:], nkw[:])
            ct0 = []
            for i in range(4):
                nc.sync.dma_start(wkv_sb[:, i * 4 * 256:(i + 1) * 4 * 256],
                                  wkv[:, 4 * i:4 * (i + 1), :])
                for hp in (2 * i, 2 * i + 1):
                    t = stream.tile([P, 2 * 512], F32R, name="st", tag="stream")
                    nc.sync.dma_start(
                        t[:], cT[:, 2 * hp:2 * hp + 2, 0:512])
                    ct0.append(t)

            # ---- constants (compute while first DMAs fly) ----
            ones_f = consts.tile([P, P], F32)
            nc.vector.memset(ones_f[:], 1.0)
            ones_r = consts.tile([P, P], F32R)
            nc.scalar.copy(ones_r[:], ones_f[:])
            ones_h = consts.tile([P, P], F16)
            nc.scalar.copy(ones_h[:], ones_f[:])
            ident_f = consts.tile([P, P], F32)
            make_identity(nc, ident_f)
            ident_h = consts.tile([P, P], F16)
            nc.scalar.copy(ident_h[:], ident_f[:])
            eps_sb = consts.tile([P, 1], F32)
            nc.vector.memset(eps_sb[:], EPS)

            nc.sync.dma_start(wq_sb[:], wq[:, :, :])
            xt0 = dma_stream(xT, 0)
            ct1 = dma_stream(cT, 1)

            def emit_b(kcol, ct_tiles):
                """K/V projection + k-rmsnorm + V transpose for one key col."""
                kvp = psum.tile([P, 1024], F32, name="kvp", tag="big", bufs=2)
                for hc in range(HC):
                    src = ct_tiles[hc // 2][:, (hc % 2) * 512:(hc % 2 + 1) * 512]
                    nc.tensor.matmul(kvp[:, 0:512],
                                     wkv_sb[:, hc * 256:hc * 256 + 128],
                                     src, start=(hc == 0), stop=(hc == HC - 1))
                    nc.tensor.matmul(kvp[:, 512:1024],
                                     wkv_sb[:, hc * 256 + 128:hc * 256 + 256],
                                     src, start=(hc == 0), stop=(hc == HC - 1))
                # v -> f16 -> transpose 128x128 blocks into v_sb [keys, D]
                vT = small.tile([P, 512], F16, name="vT", tag="vT")
                nc.vector.tensor_copy(vT[:], kvp[:, 512:1024])
                for j in range(4):
                    kt = kcol * 4 + j
                    tp = psum.tile([P, P], F16, name="tp", tag="att", bufs=4)
                    nc.tensor.transpose(tp[:], vT[:, j * P:(j + 1) * P],
                                        ident_h[:])
                    nc.vector.tensor_copy(v_sb[:, kt * D:(kt + 1) * D], tp[:])
                # k rmsnorm over D (partition dim): sumsq via ones matmul
                ksq = small.tile([P, 512], F32R, name="ksq", tag="sq")
                nc.scalar.square(ksq[:], kvp[:, 0:512])
                ksum = psum.tile([P, 512], F32, name="ksum", tag="att", bufs=4)
                nc.tensor.matmul(ksum[:], ones_r[:], ksq[:],
                                 start=True, stop=True)
                krs = small.tile([P, 512], F32, name="krs", tag="rs")
                nc.scalar.activation(krs[:], ksum[:], AF.Sqrt,
                                     bias=eps_sb[:], scale=1.0 / D)
                krr = small.tile([P, 512], F32, name="krr", tag="rr")
                nc.vector.reciprocal_approx_fast(out=krr[:], in_=krs[:])
                nc.vector.scalar_tensor_tensor(
                    out=kT_sb[:, kcol * 512:(kcol + 1) * 512], in0=kvp[:, 0:512],
                    scalar=nkw_sb[:], in1=krr[:], op0=OP.mult, op1=OP.mult)

            def emit_a(pb, xt_tiles):
                """Q projection + q-rmsnorm for one 512-query block."""
                qpairs = [psum.tile([P, 1024], F32, name=f"qp{pr}", tag="big",
                                    bufs=2) for pr in range(2)]
                for hc in range(HC):
                    src = xt_tiles[hc // 2][:, (hc % 2) * 512:(hc % 2 + 1) * 512]
                    for h in range(NH):
                        nc.tensor.matmul(
                            qpairs[h // 2][:, (h % 2) * 512:(h % 2 + 1) * 512],
                            wq_sb[:, hc * 512 + h * D:hc * 512 + (h + 1) * D],
                            src, start=(hc == 0), stop=(hc == HC - 1))
                for pr in range(2):
                    qp = qpairs[pr]
                    qsq = small.tile([P, 1024], F32R, name="qsq", tag="sq2")
                    nc.scalar.square(qsq[:], qp[:])
                    for hh in range(2):
                        h = 2 * pr + hh
                        qsum = psum.tile([P, 512], F32, name="qsum", tag="att",
                                         bufs=4)
                        nc.tensor.matmul(qsum[:], ones_r[:],
                                         qsq[:, hh * 512:(hh + 1) * 512],
                                         start=True, stop=True)
                        qrs = small.tile([P, 512], F32, name="qrs", tag="rs2")
                        nc.scalar.activation(qrs[:], qsum[:], AF.Sqrt,
                                             bias=eps_sb[:], scale=1.0 / D)
                        qrr = small.tile([P, 512], F32, name="qrr", tag="rr2")
                        nc.vector.reciprocal_approx_fast(out=qrr[:], in_=qrs[:])
                        nc.vector.scalar_tensor_tensor(
                            out=xqT_list[h][:, pb * 512:(pb + 1) * 512],
                            in0=qp[:, hh * 512:(hh + 1) * 512],
                            scalar=nqw_sb[:], in1=qrr[:],
                            op0=OP.mult, op1=OP.mult)

            def stp_exp(ab, hg, kt):
                """Score pair matmuls + exp for (q-block ab, head group hg)."""
                q0 = ab * 512
                h0, h1 = 2 * hg, 2 * hg + 1
                stp = psum.tile([P, 1024], F32, name="stp", tag="big", bufs=2)
                nc.tensor.matmul(stp[:, 0:512],
                                 kT_sb[:, kt * P:(kt + 1) * P],
                                 xqT_list[h0][:, q0:q0 + 512],
                                 start=True, stop=True)
                nc.tensor.matmul(stp[:, 512:1024],
                                 kT_sb[:, kt * P:(kt + 1) * P],
                                 xqT_list[h1][:, q0:q0 + 512],
                                 start=True, stop=True)
                es = esbp.tile([P, 1024], F16, name="es", tag="es")
                nc.scalar.activation(es[:], stp[:], AF.Exp)
                return es

            PIPE = 2  # P@V lags stp/exp by this many key-chunks

            def emit_c(ab, head):
                """Attention + wo for one 512-query block.

                head: pre-emitted es tiles for (hg=0, kt=0..PIPE-1), or None.
                Returns the es head for the next q-block (emitted before wo).
                """
                attn_map = {}
                pending_post = None
                for hg in range(2):
                    h0, h1 = 2 * hg, 2 * hg + 1
                    att0 = psum.tile([P, 512], F32, name=f"att{h0}",
                                     tag="att", bufs=4)
                    att1 = psum.tile([P, 512], F32, name=f"att{h1}",
                                     tag="att", bufs=4)
                    es_tiles = [None] * KC
                    if hg == 0 and head is not None:
                        es_tiles[:PIPE] = head
                    acc = None
                    for kt in range(KC + PIPE):
                        if kt < KC and es_tiles[kt] is None:
                            es_tiles[kt] = stp_exp(ab, hg, kt)
                        if kt == PIPE and pending_post is not None:
                            pending_post()
                            pending_post = None
                        if PIPE <= kt < KC + PIPE:
                            pes = es_tiles[kt - PIPE]
                            kb = (kt - PIPE) * D
                            nc.tensor.matmul(att0[:], v_sb[:, kb:kb + D],
                                             pes[:, 0:512],
                                             start=(kt == PIPE),
                                             stop=(kt == KC + PIPE - 1))
                            nc.tensor.matmul(att1[:], v_sb[:, kb:kb + D],
                                             pes[:, 512:1024],
                                             start=(kt == PIPE),
                                             stop=(kt == KC + PIPE - 1))
                        if kt == 1:
                            acc = accp.tile([P, 1024], F16, name="acc",
                                            tag="acc")
                            nc.vector.tensor_tensor(
                                out=acc[:], in0=es_tiles[0][:],
                                in1=es_tiles[1][:], op=OP.add)
                        elif 1 < kt < KC:
                            nacc = accp.tile([P, 1024], F16, name="acc",
                                             tag="acc")
                            nc.vector.tensor_tensor(
                                out=nacc[:], in0=acc[:], in1=es_tiles[kt][:],
                                op=OP.add)
                            acc = nacc
                    sumps = []
                    for hh in range(2):
                        sump = psum.tile([P, 512], F32, name=f"sump{hh}",
                                         tag="att", bufs=4)
                        nc.tensor.matmul(sump[:], ones_h[:],
                                         acc[:, hh * 512:(hh + 1) * 512],
                                         start=True, stop=True)
                        sumps.append(sump)

                    def post(hg=hg, atts=(att0, att1), sumps=sumps):
                        for hh, att in enumerate(atts):
                            h = 2 * hg + hh
                            rr = small.tile([P, 512], F32, name="rr",
                                            tag=f"rrc{hh}")
                            nc.vector.reciprocal_approx_fast(
                                out=rr[:], in_=sumps[hh][:])
                            attn = attnp.tile([P, 512], F32R, name=f"attn{h}",
                                              tag=f"attn{h}")
                            nc.vector.tensor_tensor(
                                out=attn[:], in0=att[:], in1=rr[:], op=OP.mult)
                            attn_map[h] = attn

                    pending_post = post
                pending_post()

                # head start for the next q-block (hides the wo wait chain)
                next_head = None
                if ab + 1 < NB:
                    next_head = [stp_exp(ab + 1, 0, kt) for kt in range(PIPE)]

                # wo: out[q, :] += attn_h^T @ wo_h, 128-row q-subtiles
                q0 = ab * 512
                for qs in range(4):
                    wps = [psum.tile([P, 1024], F32, name=f"wp{pr}", tag="big",
                                     bufs=2) for pr in range(2)]
                    for h in range(NH):
                        lhs = attn_map[h][:, qs * P:(qs + 1) * P]
                        for ht in range(4):
                            nc.tensor.matmul(
                                wps[ht // 2][:, (ht % 2) * 512:(ht % 2 + 1) * 512],
                                lhs,
                                wo_sb[:, h * HID + ht * 512:h * HID + (ht + 1) * 512],
                                start=(h == 0), stop=(h == NH - 1))
                    ot = outp.tile([P, 2048], F32, name="ot", tag="ot")
                    nc.vector.tensor_copy(ot[:, 0:1024], wps[0][:])
                    nc.scalar.copy(ot[:, 1024:2048], wps[1][:])
                    nc.gpsimd.dma_start(
                        out[q0 + qs * P:q0 + (qs + 1) * P, :], ot[:])
                return next_head

            # ======== schedule ========
            # wire order: wkv+ct0, wq, xt0, ct1, ct2, ct3, xt1, wo, xt2, xt3
            # PE order:   B0, A0, B1, B2, B3, A1, C0, A2, C1, A3, C2, C3
            emit_b(0, ct0)
            emit_a(0, xt0)
            emit_b(1, ct1)

            ct2 = dma_stream(cT, 2)
            ct3 = dma_stream(cT, 3)
            emit_b(2, ct2)
            xt1 = dma_stream(xT, 1)
            nc.sync.dma_start(wo_sb[:], wo[:, :, :])
            emit_b(3, ct3)
            emit_a(1, xt1)

            head = emit_c(0, None)
            emit_a(2, dma_stream(xT, 2))
            head = emit_c(1, head)
            emit_a(3, dma_stream(xT, 3))
            head = emit_c(2, head)
            emit_c(3, head)

    nc.compile()
    return nc


def _get_compiled():
    global _compiled
    if _compiled is None:
        _compiled = _build()
    return _compiled


def _to3d(a):
    """[rows=HC*P, cols] -> [P, HC, cols] (partition-major chunks)."""
    cols = a.shape[1]
    return np.ascontiguousarray(
        a.reshape(HC, P, cols).transpose(1, 0, 2))


def _shard_inputs(x, c, wq, wkv, wo, norm_q_w, norm_k_w):
    x = np.asarray(x, np.float32)
    c = np.asarray(c, np.float32)
    wq = np.asarray(wq, np.float32)
    wkv = np.asarray(wkv, np.float32)
    wo = np.asarray(wo, np.float32)
    nqw = (np.asarray(norm_q_w, np.float32) * np.float32(SCALE)).reshape(P, 1)
    nkw = np.asarray(norm_k_w, np.float32).reshape(P, 1).copy()

    xTs = [_to3d(np.ascontiguousarray(x[b].T)) for b in range(B)]
    cTs = [_to3d(np.ascontiguousarray(c[b].T)) for b in range(B)]
    in_maps = []
    for core in range(8):
        b, g = core // 4, core % 4
        blk = wkv[:, g * 256:(g + 1) * 256]
        kvpack = np.concatenate([blk[:, 0::2], blk[:, 1::2]], axis=1)
        wo_g = wo[g * 512:(g + 1) * 512, :]
        in_maps.append({
            "xT": xTs[b],
            "cT": cTs[b],
            "wq": _to3d(wq[:, g * 512:(g + 1) * 512]),
            "wkv": _to3d(kvpack),
            "wo": np.ascontiguousarray(
                wo_g.reshape(NH, P, HID).transpose(1, 0, 2)),
            "nqw": nqw,
            "nkw": nkw,
        })
    return in_maps


def run_sharded(inputs, trace=False, trace_cores=None):
    """Run the SPMD kernel; returns (full_output, BassKernelResults)."""
    nc = _get_compiled()
    in_maps = _shard_inputs(**inputs)
    res = run_bass_kernel_spmd(nc, in_maps, core_ids=list(range(8)),
                               trace=trace, trace_cores=trace_cores)
    parts = [r["out"] for r in res.results]
    full = np.empty((B, S, HID), np.float32)
    for b in range(B):
        full[b] = np.sum(np.stack([parts[4 * b + g] for g in range(4)], 0),
                         axis=0, dtype=np.float64).astype(np.float32)
    return full, res


def kernel(**inputs) -> np.ndarray:
    out, _ = run_sharded(inputs, trace=False)
    return out


# revision 13
# speedup vs baseline: 1.0894x; 1.0894x over previous
"""Trainium2 Bass kernel for nn_CrossAttention_51539607552970.

Sharding: 8 cores = 2 (batch) x 4 (GQA kv-head groups). Each core computes
4 query heads + its single kv head for one batch element, producing a
partial output (its head-group's contribution through wo); the host sums
the 4 partials per batch element (tensor-parallel unshard).

v3:
- Consolidated, ordered DMAs (3-D DRAM tensors; wire delivers what the PE
  needs next; weight loads split/interleaved so the first matmul starts
  within a few us).
- fp16 activations (kT / v / xq / es).
- Softmax denominator accumulated across key-chunks on the DVE (fp16 2x)
  + ONE ones-matmul per (head-pair, q-block) instead of 16.
- Paired [128,1024] 2-bank PSUM tiles halve Scalar exp count.
- Software pipelining across head-group and q-block boundaries: the next
  block's score matmuls + exp are emitted before the previous block's
  normalize/wo drain, so the PE never waits on the vector chain.
- Output copies split vector/scalar; squares moved to the DVE to cut
  activation-table churn.
"""

import sys

sys.path.insert(0, "/opt/trn_rl_repo")

import numpy as np

import concourse.bass as bass
import concourse.mybir as mybir
import concourse.tile as tile
from concourse import bacc
from concourse.bass_utils import run_bass_kernel_spmd
from concourse.masks import make_identity

F32 = mybir.dt.float32
F32R = mybir.dt.float32r
F16 = mybir.dt.float16
AF = mybir.ActivationFunctionType
OP = mybir.AluOpType

# Problem constants (hardcoded per contract).
B, S, L = 2, 2048, 2048
H, KVH, D = 16, 4, 128
HID = H * D
EPS = 1e-6
SCALE = 1.0 / np.sqrt(D)

NH = 4           # query heads per core
P = 128          # partitions
HC = HID // P    # 16 hid chunks
KC = L // P      # 16 key chunks
NB = 4           # 512-wide blocks per 2048 (both keys and queries)

_compiled = None


def _build():
    nc = bacc.Bacc("TRN2", num_devices=8)

    xT = nc.dram_tensor("xT", [P, HC, S], F32R, kind="ExternalInput")
    cT = nc.dram_tensor("cT", [P, HC, L], F32R, kind="ExternalInput")
    wq = nc.dram_tensor("wq", [P, HC, NH * D], F32R, kind="ExternalInput")
    wkv = nc.dram_tensor("wkv", [P, HC, 2 * D], F32R, kind="ExternalInput")
    wo = nc.dram_tensor("wo", [P, NH, HID], F32R, kind="ExternalInput")
    nqw = nc.dram_tensor("nqw", [P, 1], F32, kind="ExternalInput")
    nkw = nc.dram_tensor("nkw", [P, 1], F32, kind="ExternalInput")
    out = nc.dram_tensor("out", [S, HID], F32, kind="ExternalOutput")

    with nc.allow_low_precision(reason="f32r/f16 matmul input rounding"), \
         tile.TileContext(nc) as tc:
        with tc.tile_pool(name="consts", bufs=1) as consts, \
             tc.tile_pool(name="weights", bufs=1) as weights, \
             tc.tile_pool(name="stream", bufs=12) as stream, \
             tc.tile_pool(name="kv", bufs=1) as kvpool, \
             tc.tile_pool(name="xqt", bufs=1) as xqtpool, \
             tc.tile_pool(name="small", bufs=1) as small, \
             tc.tile_pool(name="esbp", bufs=3) as esbp, \
             tc.tile_pool(name="accp", bufs=2) as accp, \
             tc.tile_pool(name="attnp", bufs=1) as attnp, \
             tc.tile_pool(name="outp", bufs=2) as outp, \
             tc.tile_pool(name="psum", bufs=1, space="PSUM") as psum:

            # ---- constants ----
            ones_f = consts.tile([P, P], F32)
            nc.vector.memset(ones_f[:], 1.0)
            ones_r = consts.tile([P, P], F32R)
            nc.scalar.copy(ones_r[:], ones_f[:])
            ones_h = consts.tile([P, P], F16)
            nc.scalar.copy(ones_h[:], ones_f[:])
            ident_f = consts.tile([P, P], F32)
            make_identity(nc, ident_f)
            ident_h = consts.tile([P, P], F16)
            nc.scalar.copy(ident_h[:], ident_f[:])
            nqw_sb = consts.tile([P, 1], F32)
            nc.sync.dma_start(nqw_sb[:], nqw[:])
            nkw_sb = consts.tile([P, 1], F32)
            nc.sync.dma_start(nkw_sb[:], nkw[:])
            eps_sb = consts.tile([P, 1], F32)
            nc.vector.memset(eps_sb[:], EPS)

            # ---- resident weights (DMAs issued at scheduled points) ----
            wq_sb = weights.tile([P, HC * NH * D], F32R)
            wkv_sb = weights.tile([P, HC * 2 * D], F32R)
            wo_sb = weights.tile([P, NH * HID], F32R)

            # ---- persistent activations ----
            kT_sb = kvpool.tile([P, L], F16)              # [D, keys]
            v_sb = kvpool.tile([P, KC * D], F16)          # kt-th block = [keys, D]
            xqT_list = [xqtpool.tile([P, S], F16, name=f"xqT{h}")
                        for h in range(NH)]

            def dma_stream(src, blk):
                tiles = []
                for hp in range(HC // 2):
                    t = stream.tile([P, 2 * 512], F32R, name="st", tag="stream")
                    nc.sync.dma_start(
                        t[:], src[:, 2 * hp:2 * hp + 2,
                                  blk * 512:(blk + 1) * 512])
                    tiles.append(t)
                return tiles

            def emit_b(kcol, ct_tiles):
                """K/V projection + k-rmsnorm + V transpose for one key col."""
                kvp = psum.tile([P, 1024], F32, name="kvp", tag="big", bufs=3)
                for hc in range(HC):
                    src = ct_tiles[hc // 2][:, (hc % 2) * 512:(hc % 2 + 1) * 512]
                    nc.tensor.matmul(kvp[:, 0:512],
                                     wkv_sb[:, hc * 256:hc * 256 + 128],
                                     src, start=(hc == 0), stop=(hc == HC - 1))
                    nc.tensor.matmul(kvp[:, 512:1024],
                                     wkv_sb[:, hc * 256 + 128:hc * 256 + 256],
                                     src, start=(hc == 0), stop=(hc == HC - 1))
                # k rmsnorm over D (partition dim): sumsq via ones matmul
                ksq = small.tile([P, 512], F32R, name="ksq", tag="sq")
                nc.scalar.square(ksq[:], kvp[:, 0:512])
                ksum = psum.tile([P, 512], F32, name="ksum", tag="att", bufs=2)
                nc.tensor.matmul(ksum[:], ones_r[:], ksq[:],
                                 start=True, stop=True)
                krs = small.tile([P, 512], F32, name="krs", tag="rs")
                nc.scalar.activation(krs[:], ksum[:], AF.Sqrt,
                                     bias=eps_sb[:], scale=1.0 / D)
                krr = small.tile([P, 512], F32, name="krr", tag="rr")
                nc.vector.reciprocal_approx_fast(out=krr[:], in_=krs[:])
                nc.vector.scalar_tensor_tensor(
                    out=kT_sb[:, kcol * 512:(kcol + 1) * 512], in0=kvp[:, 0:512],
                    scalar=nkw_sb[:], in1=krr[:], op0=OP.mult, op1=OP.mult)
                # v -> f16 -> transpose 128x128 blocks into v_sb [keys, D]
                vT = small.tile([P, 512], F16, name="vT", tag="vT")
                nc.vector.tensor_copy(vT[:], kvp[:, 512:1024])
                for j in range(4):
                    kt = kcol * 4 + j
                    tp = psum.tile([P, P], F16, name="tp", tag="att", bufs=2)
                    nc.tensor.transpose(tp[:], vT[:, j * P:(j + 1) * P],
                                        ident_h[:])
                    nc.vector.tensor_copy(v_sb[:, kt * D:(kt + 1) * D], tp[:])

            def emit_a(pb, xt_tiles):
                """Q projection + q-rmsnorm for one 512-query block."""
                qpairs = [psum.tile([P, 1024], F32, name=f"qp{pr}", tag="big",
                                    bufs=3) for pr in range(2)]
                for hc in range(HC):
                    src = xt_tiles[hc // 2][:, (hc % 2) * 512:(hc % 2 + 1) * 512]
                    for h in range(NH):
                        nc.tensor.matmul(
                            qpairs[h // 2][:, (h % 2) * 512:(h % 2 + 1) * 512],
                            wq_sb[:, hc * 512 + h * D:hc * 512 + (h + 1) * D],
                            src, start=(hc == 0), stop=(hc == HC - 1))
                for pr in range(2):
                    qp = qpairs[pr]
                    qsq = small.tile([P, 1024], F32R, name="qsq", tag="sq2")
                    nc.scalar.square(qsq[:], qp[:])
                    qsum = psum.tile([P, 1024], F32, name="qsum", tag="big",
                                     bufs=3)
                    nc.tensor.matmul(qsum[:, 0:512], ones_r[:], qsq[:, 0:512],
                                     start=True, stop=True)
                    nc.tensor.matmul(qsum[:, 512:1024], ones_r[:],
                                     qsq[:, 512:1024], start=True, stop=True)
                    qrs = small.tile([P, 1024], F32, name="qrs", tag="rs2")
                    nc.scalar.activation(qrs[:], qsum[:], AF.Sqrt,
                                         bias=eps_sb[:], scale=1.0 / D)
                    qrr = small.tile([P, 1024], F32, name="qrr", tag="rr2")
                    nc.vector.reciprocal_approx_fast(out=qrr[:], in_=qrs[:])
                    for hh in range(2):
                        h = 2 * pr + hh
                        nc.vector.scalar_tensor_tensor(
                            out=xqT_list[h][:, pb * 512:(pb + 1) * 512],
                            in0=qp[:, hh * 512:(hh + 1) * 512],
                            scalar=nqw_sb[:], in1=qrr[:, hh * 512:(hh + 1) * 512],
                            op0=OP.mult, op1=OP.mult)

            def stp_exp(ab, hg, kt):
                """Score pair matmuls + exp for (q-block ab, head group hg)."""
                q0 = ab * 512
                h0, h1 = 2 * hg, 2 * hg + 1
                stp = psum.tile([P, 1024], F32, name="stp", tag="big", bufs=3)
                nc.tensor.matmul(stp[:, 0:512],
                                 kT_sb[:, kt * P:(kt + 1) * P],
                                 xqT_list[h0][:, q0:q0 + 512],
                                 start=True, stop=True)
                nc.tensor.matmul(stp[:, 512:1024],
                                 kT_sb[:, kt * P:(kt + 1) * P],
                                 xqT_list[h1][:, q0:q0 + 512],
                                 start=True, stop=True)
                es = esbp.tile([P, 1024], F16, name="es", tag="es")
                nc.scalar.activation(es[:], stp[:], AF.Exp)
                return es

            PIPE = 2  # P@V lags stp/exp by this many key-chunks

            def emit_c(ab, head):
                """Attention + wo for one 512-query block.

                head: pre-emitted es tiles for (hg=0, kt=0..PIPE-1), or None.
                Returns the es head for the next q-block (emitted before wo).
                """
                attn_map = {}
                pending_post = None
                for hg in range(2):
                    h0, h1 = 2 * hg, 2 * hg + 1
                    att0 = psum.tile([P, 512], F32, name=f"att{h0}",
                                     tag="att", bufs=2)
                    att1 = psum.tile([P, 512], F32, name=f"att{h1}",
                                     tag="att", bufs=2)
                    es_tiles = [None] * KC
                    if hg == 0 and head is not None:
                        es_tiles[:PIPE] = head
                    acc = None
                    for kt in range(KC + PIPE):
                        if kt < KC and es_tiles[kt] is None:
                            es_tiles[kt] = stp_exp(ab, hg, kt)
                        if kt == PIPE and pending_post is not None:
                            pending_post()
                            pending_post = None
                        if PIPE <= kt < KC + PIPE:
                            pes = es_tiles[kt - PIPE]
                            kb = (kt - PIPE) * D
                            nc.tensor.matmul(att0[:], v_sb[:, kb:kb + D],
                                             pes[:, 0:512],
                                             start=(kt == PIPE),
                                             stop=(kt == KC + PIPE - 1))
                            nc.tensor.matmul(att1[:], v_sb[:, kb:kb + D],
                                             pes[:, 512:1024],
                                             start=(kt == PIPE),
                                             stop=(kt == KC + PIPE - 1))
                        if kt == 1:
                            acc = accp.tile([P, 1024], F16, name="acc",
                                            tag="acc")
                            nc.vector.tensor_tensor(
                                out=acc[:], in0=es_tiles[0][:],
                                in1=es_tiles[1][:], op=OP.add)
                        elif 1 < kt < KC:
                            nacc = accp.tile([P, 1024], F16, name="acc",
                                             tag="acc")
                            nc.vector.tensor_tensor(
                                out=nacc[:], in0=acc[:], in1=es_tiles[kt][:],
                                op=OP.add)
                            acc = nacc
                    sump = psum.tile([P, 1024], F32, name="sump", tag="big",
                                     bufs=3)
                    nc.tensor.matmul(sump[:, 0:512], ones_h[:], acc[:, 0:512],
                                     start=True, stop=True)
                    nc.tensor.matmul(sump[:, 512:1024], ones_h[:],
                                     acc[:, 512:1024], start=True, stop=True)

                    def post(hg=hg, att0=att0, att1=att1, sump=sump):
                        rr = small.tile([P, 1024], F32, name="rr", tag="rr2")
                        nc.vector.reciprocal_approx_fast(out=rr[:], in_=sump[:])
                        for hh, att in ((0, att0), (1, att1)):
                            h = 2 * hg + hh
                            attn = attnp.tile([P, 512], F32R, name=f"attn{h}",
                                              tag=f"attn{h}")
                            nc.vector.tensor_tensor(
                                out=attn[:], in0=att[:],
                                in1=rr[:, hh * 512:(hh + 1) * 512], op=OP.mult)
                            attn_map[h] = attn

                    pending_post = post
                pending_post()

                # head start for the next q-block (hides the wo wait chain)
                next_head = None
                if ab + 1 < NB:
                    next_head = [stp_exp(ab + 1, 0, kt) for kt in range(PIPE)]

                # wo: out[q, :] += attn_h^T @ wo_h, 128-row q-subtiles
                q0 = ab * 512
                for qs in range(4):
                    wps = [psum.tile([P, 1024], F32, name=f"wp{pr}", tag="big",
                                     bufs=3) for pr in range(2)]
                    for h in range(NH):
                        lhs = attn_map[h][:, qs * P:(qs + 1) * P]
                        for ht in range(4):
                            nc.tensor.matmul(
                                wps[ht // 2][:, (ht % 2) * 512:(ht % 2 + 1) * 512],
                                lhs,
                                wo_sb[:, h * HID + ht * 512:h * HID + (ht + 1) * 512],
                                start=(h == 0), stop=(h == NH - 1))
                    ot = outp.tile([P, 2048], F32, name="ot", tag="ot")
                    nc.vector.tensor_copy(ot[:, 0:1024], wps[0][:])
                    nc.scalar.copy(ot[:, 1024:2048], wps[1][:])
                    nc.sync.dma_start(
                        out[q0 + qs * P:q0 + (qs + 1) * P, :], ot[:])
                return next_head

            # ======== schedule ========
            # wire order: wkv+ct0 interleaved, wq, xt0, ct1, ct2, ct3, xt1,
            #             wo, xt2, xt3
            # PE order:   B0, A0, B1, B2, B3, A1, C0, A2, C1, A3, C2, C3
            ct0 = []
            for i in range(4):
                nc.sync.dma_start(wkv_sb[:, i * 4 * 256:(i + 1) * 4 * 256],
                                  wkv[:, 4 * i:4 * (i + 1), :])
                for hp in (2 * i, 2 * i + 1):
                    t = stream.tile([P, 2 * 512], F32R, name="st", tag="stream")
                    nc.sync.dma_start(
                        t[:], cT[:, 2 * hp:2 * hp + 2, 0:512])
                    ct0.append(t)
            nc.sync.dma_start(wq_sb[:], wq[:, :, :])
            xt0 = dma_stream(xT, 0)
            ct1 = dma_stream(cT, 1)

            emit_b(0, ct0)
            emit_a(0, xt0)
            emit_b(1, ct1)

            ct2 = dma_stream(cT, 2)
            ct3 = dma_stream(cT, 3)
            emit_b(2, ct2)
            xt1 = dma_stream(xT, 1)
            nc.sync.dma_start(wo_sb[:], wo[:, :, :])
            emit_b(3, ct3)
            emit_a(1, xt1)

            head = emit_c(0, None)
            emit_a(2, dma_stream(xT, 2))
            head = emit_c(1, head)
            emit_a(3, dma_stream(xT, 3))
            head = emit_c(2, head)
            emit_c(3, head)

    nc.compile()
    return nc


def _get_compiled():
    global _compiled
    if _compiled is None:
        _compiled = _build()
    return _compiled


def _to3d(a):
    """[rows=HC*P, cols] -> [P, HC, cols] (partition-major chunks)."""
    cols = a.shape[1]
    return np.ascontiguousarray(
        a.reshape(HC, P, cols).transpose(1, 0, 2))


def _shard_inputs(x, c, wq, wkv, wo, norm_q_w, norm_k_w):
    x = np.asarray(x, np.float32)
    c = np.asarray(c, np.float32)
    wq = np.asarray(wq, np.float32)
    wkv = np.asarray(wkv, np.float32)
    wo = np.asarray(wo, np.float32)
    nqw = (np.asarray(norm_q_w, np.float32) * np.float32(SCALE)).reshape(P, 1)
    nkw = np.asarray(norm_k_w, np.float32).reshape(P, 1).copy()

    xTs = [_to3d(np.ascontiguousarray(x[b].T)) for b in range(B)]
    cTs = [_to3d(np.ascontiguousarray(c[b].T)) for b in range(B)]
    in_maps = []
    for core in range(8):
        b, g = core // 4, core % 4
        blk = wkv[:, g * 256:(g + 1) * 256]
        kvpack = np.concatenate([blk[:, 0::2], blk[:, 1::2]], axis=1)
        wo_g = wo[g * 512:(g + 1) * 512, :]
        in_maps.append({
            "xT": xTs[b],
            "cT": cTs[b],
            "wq": _to3d(wq[:, g * 512:(g + 1) * 512]),
            "wkv": _to3d(kvpack),
            "wo": np.ascontiguousarray(
                wo_g.reshape(NH, P, HID).transpose(1, 0, 2)),
            "nqw": nqw,
            "nkw": nkw,
        })
    return in_maps


def run_sharded(inputs, trace=False, trace_cores=None):
    """Run the SPMD kernel; returns (full_output, BassKernelResults)."""
    nc = _get_compiled()
    in_maps = _shard_inputs(**inputs)
    res = run_bass_kernel_spmd(nc, in_maps, core_ids=list(range(8)),
                               trace=trace, trace_cores=trace_cores)
    parts = [r["out"] for r in res.results]
    full = np.empty((B, S, HID), np.float32)
    for b in range(B):
        full[b] = np.sum(np.stack([parts[4 * b + g] for g in range(4)], 0),
                         axis=0, dtype=np.float64).astype(np.float32)
    return full, res


def kernel(**inputs) -> np.ndarray:
    out, _ = run_sharded(inputs, trace=False)
    return out
